# revision 23
# baseline (speedup 1.0000x reference)
# Trainium2 Bass kernel for nn_Decoder (RNN decoder):
#   xp = x @ W_ih^T + b_ih + b_hh            (GEMM1, bf16)
#   h_t = tanh(xp_t + h_{t-1} @ W_hh^T)      (512-step recurrence, bf16 matmul)
#   y  = hs @ W_ff^T + b_ff                  (GEMM2, bf16)
#
# Sharding: data-parallel over batch, 8 batch rows per core, weights replicated.
# Layouts are hidden-on-partitions so the sequential recurrence needs no
# transposes: h is stored [hid(4x128 part-tiles), batch(8)].
#
# Recurrence structure per step (critical path):
#   - output hid-tile m=0 accumulates in its own PSUM tensor z0 (1 bank),
#     tiles m=1..3 in zB (3 banks, double-buffered) — separate tensors so the
#     dependency tracker never serializes PE writes against the other half's
#     tanh read.
#   - per bank: identity-matmul injects xp_t (start=True, dep-free, hoisted
#     into PE idle), then 4 W_hh^T k-tile matmuls accumulate W@h.
#   - tanh split 3+1 on ACT: actB (tiles 1-3) is the critical producer and is
#     scheduled first; actA (tile 0) trails and feeds the next step's k=0
#     matmuls, which are ordered last in the burst.
#   - h history is split across 4 tensors by (step parity) x (A/B half) for
#     minimal tracker deps; GEMM2 consumes all four.
#   - most of GEMM2 is interleaved into the recurrence's PE idle (1 matmul
#     per step from t=384), with drains on the otherwise-idle DVE.

import numpy as np
import ml_dtypes

B, S, I, H, O = 64, 512, 256, 512, 256
NCORES = 8
BL = B // NCORES  # 8 batch rows per core
P = 128
KI, KH, KO = I // P, H // P, O // P  # 2, 4, 2
CH = 512                             # free-dim chunk for GEMM1
_builder_cache = {}


def build_nc(seq_len=S):
    """Build the (single-core SPMD) Bass program for sequence length seq_len."""
    import concourse.bass as bass
    import concourse.mybir as mybir
    import concourse.tile as tile
    from concourse import bacc

    f32 = mybir.dt.float32
    bf16 = mybir.dt.bfloat16
    AF = mybir.ActivationFunctionType

    s = seq_len
    assert s % 2 == 0
    F = s * BL               # free length of (t, b) axis
    nch = max(1, F // CH)    # chunks for GEMM1
    ch = F // nch
    F2 = F // 2              # per-parity free length for GEMM2
    CH2 = 256
    nch2 = max(1, F2 // CH2)
    ch2 = F2 // nch2

    nc = bacc.Bacc("TRN2")

    xt = nc.dram_tensor("xt", [I, F], bf16, kind="ExternalInput")      # x^T  (in, t*BL+b)
    h0t = nc.dram_tensor("h0t", [H, BL], bf16, kind="ExternalInput")   # h0^T (hid, b)
    wih = nc.dram_tensor("wih", [I, H], bf16, kind="ExternalInput")    # W_ih^T
    whh = nc.dram_tensor("whh", [H, H], bf16, kind="ExternalInput")    # W_hh^T
    wff = nc.dram_tensor("wff", [H, O], bf16, kind="ExternalInput")    # W_ff^T
    bcb = nc.dram_tensor("bcb", [P, KH], f32, kind="ExternalInput")    # b_ih+b_hh, [128, 4]
    bfb = nc.dram_tensor("bfb", [P, KO], f32, kind="ExternalInput")    # b_ff, [128, 2]
    eye = nc.dram_tensor("eye", [P, P], bf16, kind="ExternalInput")
    # y[ot, p, par, q*BL + b]:  par=0 -> t = 2q+1, par=1 -> t = 2q
    y = nc.dram_tensor("y", [KO, P, 2, F2], f32, kind="ExternalOutput")

    with tile.TileContext(nc) as tc:
        with (
            tc.tile_pool(name="const", bufs=1) as cp,
            tc.tile_pool(name="big", bufs=1) as bp,
        ):
            wih_sb = cp.tile([P, KI, H], bf16)
            whh_sb = cp.tile([P, KH, H], bf16)
            wff_sb = cp.tile([P, KH, O], bf16)
            bcb_sb = cp.tile([P, KH], f32)
            bfb_sb = cp.tile([P, KO], f32)
            eye_sb = cp.tile([P, P], bf16)

            xt_sb = bp.tile([P, KI, F], bf16)
            xp_sb = bp.tile([P, KH, F], bf16)
            # h_i (i = t+1, 0..s) lives in parity tensor (i % 2) at col-slot
            # (i // 2) * BL; the A tensor holds hid-tile 0, B holds tiles 1..3.
            n0 = (s // 2 + 1) * BL
            n1 = (s // 2) * BL
            hs0A = bp.tile([P, 1, n0], bf16)
            hs0B = bp.tile([P, KH - 1, n0], bf16)
            hs1A = bp.tile([P, 1, n1], bf16)
            hs1B = bp.tile([P, KH - 1, n1], bf16)
            hA = [hs0A, hs1A]
            hB = [hs0B, hs1B]
            out_sb = bp.tile([P, KO, 2, F2], f32)

            # ---- input loads (all bf16 host-side, plain HWDGE) ----
            # eye first: it feeds the PE warmup matmuls that run during the
            # remaining input DMAs.
            xt_r = xt[:].rearrange("(k p) f -> p k f", p=P)
            nc.sync.dma_start(eye_sb[:], eye[:])
            nc.sync.dma_start(xt_sb[:, :, 0:ch], xt_r[:, :, 0:ch])
            nc.sync.dma_start(wih_sb[:], wih[:].rearrange("(k p) h -> p k h", p=P))
            nc.sync.dma_start(bcb_sb[:], bcb[:])
            h0r = h0t[:].rearrange("(k p) b -> p k b", p=P)
            nc.sync.dma_start(hs0A[:, :, 0:BL], h0r[:, 0:1, :])
            nc.sync.dma_start(hs0B[:, :, 0:BL], h0r[:, 1:KH, :])
            nc.sync.dma_start(whh_sb[:], whh[:].rearrange("(k p) h -> p k h", p=P))
            nc.sync.dma_start(wff_sb[:], wff[:].rearrange("(k p) o -> p k o", p=P))
            nc.sync.dma_start(bfb_sb[:], bfb[:])

            # ---- GEMM1: xp[hid, (t,b)] = W_ih @ x^T + (b_ih + b_hh) ----
            # Drains alternate DVE/ACT so neither engine's drain tail idles
            # the PE long enough to matter.
            with tc.tile_pool(name="g1ps", bufs=4, space=bass.MemorySpace.PSUM) as g1p:
                # PE warmup during the input-DMA wait: N=512 matmuls run at
                # half rate until the HAM clock gate sees ~3.4us of sustained
                # PE activity, so spend the DMA-bound window warming up.
                wm = g1p.tile([P, 512], f32, tag="ps")
                for _ in range(40):
                    nc.tensor.matmul(
                        wm[:, 0:P], eye_sb[:], eye_sb[:], start=True, stop=True
                    )
                for j in range(nch):
                    sl = slice(j * ch, (j + 1) * ch)
                    if j + 1 < nch:
                        sl2 = slice((j + 1) * ch, (j + 2) * ch)
                        nc.sync.dma_start(xt_sb[:, :, sl2], xt_r[:, :, sl2])
                    for m in range(KH):
                        ps = g1p.tile([P, ch], f32)
                        for k in range(KI):
                            nc.tensor.matmul(
                                ps[:],
                                wih_sb[:, k, m * P : (m + 1) * P],
                                xt_sb[:, k, sl],
                                start=(k == 0),
                                stop=(k == KI - 1),
                            )
                        if m % 2 == 0:
                            nc.vector.tensor_scalar_add(
                                xp_sb[:, m, sl], ps[:], bcb_sb[:, m : m + 1]
                            )
                        else:
                            nc.scalar.activation(
                                xp_sb[:, m, sl], ps[:], AF.Identity,
                                bias=bcb_sb[:, m : m + 1],
                            )

            # ---- recurrence, with most of GEMM2 interleaved ----
            y_r = y[:].rearrange("o p q f -> p o q f")
            # GEMM2 work units: each is (par, j2, ot) = 4 matmuls + 1 DVE
            # drain (+ the chunk's output DMA on the last ot).
            g2_jobs = [
                (par, j2, ot)
                for j2 in range(nch2)
                for par in range(2)
                for ot in range(KO)
            ]

            def g2_emit(job, g2p):
                par, j2, ot = job
                pA, pB = hA[par], hB[par]
                base = BL if par == 0 else 0
                sl = slice(j2 * ch2, (j2 + 1) * ch2)
                hsl = slice(base + j2 * ch2, base + (j2 + 1) * ch2)
                ps = g2p.tile([P, ch2], f32, tag="g2ps")
                for k in range(KH):
                    rhs = pA[:, 0, hsl] if k == 0 else pB[:, k - 1, hsl]
                    nc.tensor.matmul(
                        ps[:],
                        wff_sb[:, k, ot * P : (ot + 1) * P],
                        rhs,
                        start=(k == 0),
                        stop=(k == KH - 1),
                    )
                nc.vector.tensor_scalar_add(
                    out_sb[:, ot, par, sl], ps[:], bfb_sb[:, ot : ot + 1]
                )
                if ot == KO - 1:
                    nc.sync.dma_start(y_r[:, :, par, sl], out_sb[:, :, par, sl])
            # job i is legal once all h-slots it reads exist: chunk j2 covers
            # t <= 64*j2 + 63, i.e. after step 64*j2 + 63.
            def g2_ready_step(job):
                par, j2, ot = job
                return 64 * (j2 + 1)

            with (
                tc.tile_pool(name="z0ps", bufs=1, space=bass.MemorySpace.PSUM) as z0p,
                tc.tile_pool(name="zBps", bufs=2, space=bass.MemorySpace.PSUM) as zBp,
                tc.tile_pool(name="g2ps", bufs=1, space=bass.MemorySpace.PSUM) as g2p,
            ):
                from concourse.tile import add_dep_helper

                g2_i = 0
                prev_last_k0 = None
                for t in range(s):
                    z0 = z0p.tile([P, 512], f32)
                    zB = zBp.tile([P, KH - 1, 512], f32)
                    rA, rB = hA[t % 2], hB[t % 2]
                    wA, wB = hA[(t + 1) % 2], hB[(t + 1) % 2]
                    rof = (t // 2) * BL
                    wof = ((t + 1) // 2) * BL

                    def kmm(m, k):
                        zt = z0[:, 0:BL] if m == 0 else zB[:, m - 1, 0:BL]
                        rhs = (
                            rA[:, 0, rof : rof + BL]
                            if k == 0
                            else rB[:, k - 1, rof : rof + BL]
                        )
                        return nc.tensor.matmul(
                            zt,
                            whh_sb[:, k, m * P : (m + 1) * P],
                            rhs,
                            start=False,
                            stop=(k == 0),
                        )

                    def imm(m):
                        zt = z0[:, 0:BL] if m == 0 else zB[:, m - 1, 0:BL]
                        return nc.tensor.matmul(
                            zt,
                            eye_sb[:],
                            xp_sb[:, m, t * BL : (t + 1) * BL],
                            start=True,
                            stop=False,
                        )

                    for m in (1, 2, 3):
                        ei = imm(m)
                        if prev_last_k0 is not None:
                            # ordering-only: keep dep-free xp-inject matmuls
                            # from being scheduled ahead of the previous
                            # step's k=0 matmuls in the PE stream
                            add_dep_helper(ei.ins, prev_last_k0.ins, sync=False)
                    for k in (1, 2, 3):
                        for m in (1, 2, 3):
                            kmm(m, k)
                    for m in (1, 2, 3):
                        prev_last_k0 = kmm(m, 0)
                    nc.scalar.activation(
                        wB[:, :, wof : wof + BL], zB[:, :, 0:BL], AF.Tanh
                    )
                    imm(0)
                    for k in (1, 2, 3, 0):
                        kmm(0, k)
                    nc.scalar.activation(
                        wA[:, 0, wof : wof + BL], z0[:, 0:BL], AF.Tanh
                    )
                    # one GEMM2 unit every few steps in the tanh shadow, once
                    # its input h-slots exist
                    if t >= 96 and t % 4 == 0 and g2_i < len(g2_jobs):
                        job = g2_jobs[g2_i]
                        if g2_ready_step(job) <= t:
                            g2_emit(job, g2p)
                            g2_i += 1
                # tail of GEMM2 (last chunks need the final steps)
                while g2_i < len(g2_jobs):
                    g2_emit(g2_jobs[g2_i], g2p)
                    g2_i += 1

    return nc


def make_in_maps(x, h0, W_ih, W_hh, b_ih, b_hh, W_ff, b_ff, seq_len=S):
    """Host-side sharding + layout prep: per-core input dicts."""
    bf = ml_dtypes.bfloat16
    x = np.asarray(x, np.float32)
    h0 = np.asarray(h0, np.float32)
    wih = np.ascontiguousarray(np.asarray(W_ih, np.float32).T).astype(bf)   # [I, H]
    whh = np.ascontiguousarray(np.asarray(W_hh, np.float32).T).astype(bf)   # [H, H]
    wff = np.ascontiguousarray(np.asarray(W_ff, np.float32).T).astype(bf)   # [H, O]
    bc = np.asarray(b_ih, np.float32) + np.asarray(b_hh, np.float32)
    bcb = np.ascontiguousarray(bc.reshape(KH, P).T)             # [128, KH]
    bfb = np.ascontiguousarray(np.asarray(b_ff, np.float32).reshape(KO, P).T)
    eye = np.eye(P, dtype=np.float32).astype(bf)

    in_maps = []
    for c in range(NCORES):
        xs = x[c * BL : (c + 1) * BL, :seq_len]                 # [BL, s, I]
        xt = np.ascontiguousarray(xs.transpose(2, 1, 0)).reshape(I, seq_len * BL)
        h0t = np.ascontiguousarray(h0[c * BL : (c + 1) * BL].T)  # [H, BL]
        in_maps.append(
            {
                "xt": xt.astype(bf),
                "h0t": h0t.astype(bf),
                "wih": wih,
                "whh": whh,
                "wff": wff,
                "bcb": bcb,
                "bfb": bfb,
                "eye": eye,
            }
        )
    return in_maps


def assemble_output(results, seq_len=S):
    """Per-core y [KO, 128, 2, (s/2)*BL] -> full [B, s, O]."""
    s = seq_len
    outs = []
    for r in results:
        yc = np.asarray(r["y"]).reshape(O, 2, s // 2, BL)
        full = np.empty((O, s, BL), np.float32)
        full[:, 1::2, :] = yc[:, 0]   # par=0: t = 2q+1
        full[:, 0::2, :] = yc[:, 1]   # par=1: t = 2q
        outs.append(full.transpose(2, 1, 0))
    return np.ascontiguousarray(np.concatenate(outs, axis=0))


def _get_finalized_nc(seq_len=S):
    key = ("nc", seq_len)
    if key not in _builder_cache:
        nc = build_nc(seq_len)
        nc.finalize()
        _builder_cache[key] = nc
    return _builder_cache[key]


def run_on_cores(inputs, seq_len=S, **kwargs):
    from concourse.bass_utils import run_bass_kernel_spmd

    nc = _get_finalized_nc(seq_len)
    in_maps = make_in_maps(**inputs, seq_len=seq_len)
    res = run_bass_kernel_spmd(nc, in_maps, core_ids=list(range(NCORES)), **kwargs)
    return res


def kernel(**inputs) -> np.ndarray:
    res = run_on_cores(inputs)
    return assemble_output(res.results)


# revision 24
# speedup vs baseline: 1.0039x; 1.0039x over previous
# Trainium2 Bass kernel for nn_Decoder (RNN decoder):
#   xp = x @ W_ih^T + b_ih + b_hh            (GEMM1, bf16)
#   h_t = tanh(xp_t + h_{t-1} @ W_hh^T)      (512-step recurrence, bf16 matmul)
#   y  = hs @ W_ff^T + b_ff                  (GEMM2, bf16)
#
# Sharding: data-parallel over batch, 8 batch rows per core, weights replicated.
# Layouts are hidden-on-partitions so the sequential recurrence needs no
# transposes: h is stored [hid(4x128 part-tiles), batch(8)].
#
# Recurrence structure per step (critical path):
#   - output hid-tile m=0 accumulates in its own PSUM tensor z0 (1 bank),
#     tiles m=1..3 in zB (3 banks, double-buffered) — separate tensors so the
#     dependency tracker never serializes PE writes against the other half's
#     tanh read.
#   - per bank: identity-matmul injects xp_t (start=True, dep-free, hoisted
#     into PE idle), then 4 W_hh^T k-tile matmuls accumulate W@h.
#   - tanh split 3+1 on ACT: actB (tiles 1-3) is the critical producer and is
#     scheduled first; actA (tile 0) trails and feeds the next step's k=0
#     matmuls, which are ordered last in the burst.
#   - h history is split across 4 tensors by (step parity) x (A/B half) for
#     minimal tracker deps; GEMM2 consumes all four.
#   - most of GEMM2 is interleaved into the recurrence's PE idle (1 matmul
#     per step from t=384), with drains on the otherwise-idle DVE.

import numpy as np
import ml_dtypes

B, S, I, H, O = 64, 512, 256, 512, 256
NCORES = 8
BL = B // NCORES  # 8 batch rows per core
P = 128
KI, KH, KO = I // P, H // P, O // P  # 2, 4, 2
CH = 512                             # free-dim chunk for GEMM1
_builder_cache = {}


def build_nc(seq_len=S):
    """Build the (single-core SPMD) Bass program for sequence length seq_len."""
    import concourse.bass as bass
    import concourse.mybir as mybir
    import concourse.tile as tile
    from concourse import bacc

    f32 = mybir.dt.float32
    bf16 = mybir.dt.bfloat16
    AF = mybir.ActivationFunctionType

    s = seq_len
    assert s % 2 == 0
    F = s * BL               # free length of (t, b) axis
    nch = max(1, F // CH)    # chunks for GEMM1
    ch = F // nch
    F2 = F // 2              # per-parity free length for GEMM2
    CH2 = 256
    nch2 = max(1, F2 // CH2)
    ch2 = F2 // nch2

    nc = bacc.Bacc("TRN2")

    xt = nc.dram_tensor("xt", [I, F], bf16, kind="ExternalInput")      # x^T  (in, t*BL+b)
    h0t = nc.dram_tensor("h0t", [H, BL], bf16, kind="ExternalInput")   # h0^T (hid, b)
    wih = nc.dram_tensor("wih", [I, H], bf16, kind="ExternalInput")    # W_ih^T
    whh = nc.dram_tensor("whh", [H, H], bf16, kind="ExternalInput")    # W_hh^T
    wff = nc.dram_tensor("wff", [H, O], bf16, kind="ExternalInput")    # W_ff^T
    bcb = nc.dram_tensor("bcb", [P, KH], f32, kind="ExternalInput")    # b_ih+b_hh, [128, 4]
    bfb = nc.dram_tensor("bfb", [P, KO], f32, kind="ExternalInput")    # b_ff, [128, 2]
    eye = nc.dram_tensor("eye", [P, P], bf16, kind="ExternalInput")
    # y[ot, p, par, q*BL + b]:  par=0 -> t = 2q+1, par=1 -> t = 2q
    y = nc.dram_tensor("y", [KO, P, 2, F2], f32, kind="ExternalOutput")

    with tile.TileContext(nc) as tc:
        with (
            tc.tile_pool(name="const", bufs=1) as cp,
            tc.tile_pool(name="big", bufs=1) as bp,
        ):
            wih_sb = cp.tile([P, KI, H], bf16)
            whh_sb = cp.tile([P, KH, H], bf16)
            wff_sb = cp.tile([P, KH, O], bf16)
            bcb_sb = cp.tile([P, KH], f32)
            bfb_sb = cp.tile([P, KO], f32)
            eye_sb = cp.tile([P, P], bf16)

            xt_sb = bp.tile([P, KI, F], bf16)
            xp_sb = bp.tile([P, KH, F], bf16)
            # h_i (i = t+1, 0..s) lives in parity tensor (i % 2) at col-slot
            # (i // 2) * BL; the A tensor holds hid-tile 0, B holds tiles 1..3.
            n0 = (s // 2 + 1) * BL
            n1 = (s // 2) * BL
            hs0A = bp.tile([P, 1, n0], bf16)
            hs0B = bp.tile([P, KH - 1, n0], bf16)
            hs1A = bp.tile([P, 1, n1], bf16)
            hs1B = bp.tile([P, KH - 1, n1], bf16)
            hA = [hs0A, hs1A]
            hB = [hs0B, hs1B]
            out_sb = bp.tile([P, KO, 2, F2], f32)

            # ---- input loads (all bf16 host-side, plain HWDGE) ----
            # eye first: it feeds the PE warmup matmuls that run during the
            # remaining input DMAs.
            xt_r = xt[:].rearrange("(k p) f -> p k f", p=P)
            nc.sync.dma_start(eye_sb[:], eye[:])
            nc.sync.dma_start(xt_sb[:, :, 0:ch], xt_r[:, :, 0:ch])
            nc.sync.dma_start(wih_sb[:], wih[:].rearrange("(k p) h -> p k h", p=P))
            nc.sync.dma_start(bcb_sb[:], bcb[:])
            h0r = h0t[:].rearrange("(k p) b -> p k b", p=P)
            nc.sync.dma_start(hs0A[:, :, 0:BL], h0r[:, 0:1, :])
            nc.sync.dma_start(hs0B[:, :, 0:BL], h0r[:, 1:KH, :])
            nc.sync.dma_start(whh_sb[:], whh[:].rearrange("(k p) h -> p k h", p=P))
            nc.sync.dma_start(wff_sb[:], wff[:].rearrange("(k p) o -> p k o", p=P))
            nc.sync.dma_start(bfb_sb[:], bfb[:])

            # ---- GEMM1: xp[hid, (t,b)] = W_ih @ x^T + (b_ih + b_hh) ----
            # Drains alternate DVE/ACT so neither engine's drain tail idles
            # the PE long enough to matter.
            with tc.tile_pool(name="g1ps", bufs=6, space=bass.MemorySpace.PSUM) as g1p:
                # PE warmup during the input-DMA wait: N=512 matmuls run at
                # half rate until the HAM clock gate sees ~3.4us of sustained
                # PE activity, so spend the DMA-bound window warming up.
                wm = g1p.tile([P, 512], f32, tag="ps")
                for _ in range(40):
                    nc.tensor.matmul(
                        wm[:, 0:P], eye_sb[:], eye_sb[:], start=True, stop=True
                    )
                for j in range(nch):
                    sl = slice(j * ch, (j + 1) * ch)
                    if j + 1 < nch:
                        sl2 = slice((j + 1) * ch, (j + 2) * ch)
                        nc.sync.dma_start(xt_sb[:, :, sl2], xt_r[:, :, sl2])
                    for m in range(KH):
                        ps = g1p.tile([P, ch], f32)
                        for k in range(KI):
                            nc.tensor.matmul(
                                ps[:],
                                wih_sb[:, k, m * P : (m + 1) * P],
                                xt_sb[:, k, sl],
                                start=(k == 0),
                                stop=(k == KI - 1),
                            )
                        if m % 2 == 0:
                            nc.vector.tensor_scalar_add(
                                xp_sb[:, m, sl], ps[:], bcb_sb[:, m : m + 1]
                            )
                        else:
                            nc.scalar.activation(
                                xp_sb[:, m, sl], ps[:], AF.Identity,
                                bias=bcb_sb[:, m : m + 1],
                            )

            # ---- recurrence, with most of GEMM2 interleaved ----
            y_r = y[:].rearrange("o p q f -> p o q f")
            # GEMM2 work units: each is (par, j2, ot) = 4 matmuls + 1 DVE
            # drain (+ the chunk's output DMA on the last ot).
            g2_jobs = [
                (par, j2, ot)
                for j2 in range(nch2)
                for par in range(2)
                for ot in range(KO)
            ]

            def g2_emit(job, g2p):
                par, j2, ot = job
                pA, pB = hA[par], hB[par]
                base = BL if par == 0 else 0
                sl = slice(j2 * ch2, (j2 + 1) * ch2)
                hsl = slice(base + j2 * ch2, base + (j2 + 1) * ch2)
                ps = g2p.tile([P, ch2], f32, tag="g2ps")
                for k in range(KH):
                    rhs = pA[:, 0, hsl] if k == 0 else pB[:, k - 1, hsl]
                    nc.tensor.matmul(
                        ps[:],
                        wff_sb[:, k, ot * P : (ot + 1) * P],
                        rhs,
                        start=(k == 0),
                        stop=(k == KH - 1),
                    )
                nc.vector.tensor_scalar_add(
                    out_sb[:, ot, par, sl], ps[:], bfb_sb[:, ot : ot + 1]
                )
                if ot == KO - 1:
                    nc.sync.dma_start(y_r[:, :, par, sl], out_sb[:, :, par, sl])
            # job i is legal once all h-slots it reads exist: chunk j2 covers
            # t <= 64*j2 + 63, i.e. after step 64*j2 + 63.
            def g2_ready_step(job):
                par, j2, ot = job
                return 64 * (j2 + 1)

            with (
                tc.tile_pool(name="z0ps", bufs=1, space=bass.MemorySpace.PSUM) as z0p,
                tc.tile_pool(name="zBps", bufs=2, space=bass.MemorySpace.PSUM) as zBp,
                tc.tile_pool(name="g2ps", bufs=1, space=bass.MemorySpace.PSUM) as g2p,
            ):
                from concourse.tile import add_dep_helper

                g2_i = 0
                prev_last_k0 = None
                for t in range(s):
                    z0 = z0p.tile([P, 512], f32)
                    zB = zBp.tile([P, KH - 1, 512], f32)
                    rA, rB = hA[t % 2], hB[t % 2]
                    wA, wB = hA[(t + 1) % 2], hB[(t + 1) % 2]
                    rof = (t // 2) * BL
                    wof = ((t + 1) // 2) * BL

                    def kmm(m, k):
                        zt = z0[:, 0:BL] if m == 0 else zB[:, m - 1, 0:BL]
                        rhs = (
                            rA[:, 0, rof : rof + BL]
                            if k == 0
                            else rB[:, k - 1, rof : rof + BL]
                        )
                        return nc.tensor.matmul(
                            zt,
                            whh_sb[:, k, m * P : (m + 1) * P],
                            rhs,
                            start=False,
                            stop=(k == 0),
                        )

                    def imm(m):
                        zt = z0[:, 0:BL] if m == 0 else zB[:, m - 1, 0:BL]
                        return nc.tensor.matmul(
                            zt,
                            eye_sb[:],
                            xp_sb[:, m, t * BL : (t + 1) * BL],
                            start=True,
                            stop=False,
                        )

                    for m in (1, 2, 3):
                        ei = imm(m)
                        if prev_last_k0 is not None:
                            # ordering-only: keep dep-free xp-inject matmuls
                            # from being scheduled ahead of the previous
                            # step's k=0 matmuls in the PE stream
                            add_dep_helper(ei.ins, prev_last_k0.ins, sync=False)
                    for k in (1, 2, 3):
                        for m in (1, 2, 3):
                            kmm(m, k)
                    for m in (1, 2, 3):
                        prev_last_k0 = kmm(m, 0)
                    nc.scalar.activation(
                        wB[:, :, wof : wof + BL], zB[:, :, 0:BL], AF.Tanh
                    )
                    imm(0)
                    for k in (1, 2, 3, 0):
                        kmm(0, k)
                    nc.scalar.activation(
                        wA[:, 0, wof : wof + BL], z0[:, 0:BL], AF.Tanh
                    )
                    # one GEMM2 unit every few steps in the tanh shadow, once
                    # its input h-slots exist
                    if t >= 96 and t % 4 == 0 and g2_i < len(g2_jobs):
                        job = g2_jobs[g2_i]
                        if g2_ready_step(job) <= t:
                            g2_emit(job, g2p)
                            g2_i += 1
                # tail of GEMM2 (last chunks need the final steps)
                while g2_i < len(g2_jobs):
                    g2_emit(g2_jobs[g2_i], g2p)
                    g2_i += 1

    return nc


def make_in_maps(x, h0, W_ih, W_hh, b_ih, b_hh, W_ff, b_ff, seq_len=S):
    """Host-side sharding + layout prep: per-core input dicts."""
    bf = ml_dtypes.bfloat16
    x = np.asarray(x, np.float32)
    h0 = np.asarray(h0, np.float32)
    wih = np.ascontiguousarray(np.asarray(W_ih, np.float32).T).astype(bf)   # [I, H]
    whh = np.ascontiguousarray(np.asarray(W_hh, np.float32).T).astype(bf)   # [H, H]
    wff = np.ascontiguousarray(np.asarray(W_ff, np.float32).T).astype(bf)   # [H, O]
    bc = np.asarray(b_ih, np.float32) + np.asarray(b_hh, np.float32)
    bcb = np.ascontiguousarray(bc.reshape(KH, P).T)             # [128, KH]
    bfb = np.ascontiguousarray(np.asarray(b_ff, np.float32).reshape(KO, P).T)
    eye = np.eye(P, dtype=np.float32).astype(bf)

    in_maps = []
    for c in range(NCORES):
        xs = x[c * BL : (c + 1) * BL, :seq_len]                 # [BL, s, I]
        xt = np.ascontiguousarray(xs.transpose(2, 1, 0)).reshape(I, seq_len * BL)
        h0t = np.ascontiguousarray(h0[c * BL : (c + 1) * BL].T)  # [H, BL]
        in_maps.append(
            {
                "xt": xt.astype(bf),
                "h0t": h0t.astype(bf),
                "wih": wih,
                "whh": whh,
                "wff": wff,
                "bcb": bcb,
                "bfb": bfb,
                "eye": eye,
            }
        )
    return in_maps


def assemble_output(results, seq_len=S):
    """Per-core y [KO, 128, 2, (s/2)*BL] -> full [B, s, O]."""
    s = seq_len
    outs = []
    for r in results:
        yc = np.asarray(r["y"]).reshape(O, 2, s // 2, BL)
        full = np.empty((O, s, BL), np.float32)
        full[:, 1::2, :] = yc[:, 0]   # par=0: t = 2q+1
        full[:, 0::2, :] = yc[:, 1]   # par=1: t = 2q
        outs.append(full.transpose(2, 1, 0))
    return np.ascontiguousarray(np.concatenate(outs, axis=0))


def _get_finalized_nc(seq_len=S):
    key = ("nc", seq_len)
    if key not in _builder_cache:
        nc = build_nc(seq_len)
        nc.finalize()
        _builder_cache[key] = nc
    return _builder_cache[key]


def run_on_cores(inputs, seq_len=S, **kwargs):
    from concourse.bass_utils import run_bass_kernel_spmd

    nc = _get_finalized_nc(seq_len)
    in_maps = make_in_maps(**inputs, seq_len=seq_len)
    res = run_bass_kernel_spmd(nc, in_maps, core_ids=list(range(NCORES)), **kwargs)
    return res


def kernel(**inputs) -> np.ndarray:
    res = run_on_cores(inputs)
    return assemble_output(res.results)
